# revision 1
# baseline (speedup 1.0000x reference)
"""Causal single-head attention (B=4, S=4096, D=1024) on 8 TRN2 NeuronCores.

Sharding: core = (batch b, half h).  Each core computes attention output for
2048 queries of one batch: query chunks {0,3,4,7} (h=0) or {1,2,5,6} (h=1) of
8x512, which balances causal work.  Each core projects K^T/V for its full
batch (Q projections zippered in between the chunks); K^T lives in SBUF as
four independently-gated fp16 tiles and V is streamed from a DRAM scratch on
the Scalar HWDGE queue.
Scores are computed in the S^T = [k, q] layout so no on-device transposes are
needed anywhere:
  K^T/Q^T/V projections:  psum = sum_d WT[d,:128].T @ x^T[d,:]      (fp16)
  scores^T[k,q]        :  psum = sum_o KT[o,k128].T @ QT[o,q512]    (fp16)
  P = exp(s*scale) * causal_mask   (mask = (iota_k - iota_q) <= a[slot,j])
  den[1,q]             :  ones[k,1].T @ P^T                         (fp16)
  ctx^T[o,q]           :  psum = sum_k V[k,o128].T @ P^T[k,q]       (fp16)
  out = ctx^T * (1/den)  broadcast via ones[1,128].T @ recip[1,q]
"""

import sys

for _p in ("/opt/trn_rl_repo",):
    if _p not in sys.path:
        sys.path.insert(0, _p)

import numpy as np

B, S, D = 4, 4096, 1024
P = 128
CH = 512                       # query chunk
NSLOT = 4                      # chunks per core
NQ = NSLOT * CH                # queries per core
NK = [8, 16, 24, 32]           # k-tiles per slot (uniform across cores)
SLOTBASE = [0, 8, 24, 48]      # amat column base per slot
CHUNKS_H = [[0, 3, 4, 7], [1, 2, 5, 6]]
SCALE = 1.0 / 32.0             # 1/sqrt(D)

_PROGRAM = None


def _build_program():
    import concourse.bass as bass
    import concourse.tile as tile
    import concourse.mybir as mybir
    from concourse import bacc
    from concourse.bass import ds, ts

    f32 = mybir.dt.float32
    f16 = mybir.dt.float16

    nc = bacc.Bacc(trn_type="TRN2", target_bir_lowering=False, debug=False,
                   num_devices=8)

    xT = nc.declare_dram_parameter("xT", [8, P, 8, CH], f16, isOutput=False)
    xqT = nc.declare_dram_parameter("xqT", [NSLOT, P, 8, CH], f16, isOutput=False)
    wqT = nc.declare_dram_parameter("wqT", [P, 8, D], f16, isOutput=False)
    wkT = nc.declare_dram_parameter("wkT", [P, 8, D], f16, isOutput=False)
    wvT = nc.declare_dram_parameter("wvT", [P, 8, D], f16, isOutput=False)
    amat = nc.declare_dram_parameter("amat", [P, 80], f16, isOutput=False)
    dmat = nc.declare_dram_parameter("dmat", [P, CH], f16, isOutput=False)
    ones_k = nc.declare_dram_parameter("ones_k", [P, 1], f16, isOutput=False)
    ones_r = nc.declare_dram_parameter("ones_r", [1, P], f32, isOutput=False)
    outT = nc.declare_dram_parameter("outT", [D, NQ], f32, isOutput=True)

    H = S // 4  # 1024: columns per resident K^T piece
    vscr = nc.dram_tensor("v_scratch", [S, D], f16)

    Exp = mybir.ActivationFunctionType.Exp
    is_le = mybir.AluOpType.is_le
    mult = mybir.AluOpType.mult

    with tile.TileContext(nc, pool_alloc_mode="queue") as tc:
        with (
            tc.tile_pool(name="kt", bufs=1) as kt_pool,
            tc.tile_pool(name="qt", bufs=1) as qt_pool,
            tc.tile_pool(name="const", bufs=1) as const_pool,
        ):
            KTp = [
                kt_pool.tile([P, 8, H], f16, tag=f"kt{i}", name=f"KTp{i}")
                for i in range(4)
            ]
            QTs = [
                qt_pool.tile([P, 8, CH], f16, tag=f"qt{i}", name=f"QTs{i}")
                for i in range(NSLOT)
            ]
            dmat_sb = const_pool.tile([P, CH], f16, tag="dmat")
            amat_sb = const_pool.tile([P, 80], f16, tag="amat")
            ones_k_sb = const_pool.tile([P, 1], f16, tag="onesk")
            ones_r_sb = const_pool.tile([1, P], f32, tag="onesr")
            nc.sync.dma_start(out=dmat_sb[:], in_=dmat[:])
            nc.sync.dma_start(out=amat_sb[:], in_=amat[:])
            nc.sync.dma_start(out=ones_k_sb[:], in_=ones_k[:])
            nc.sync.dma_start(out=ones_r_sb[:], in_=ones_r[:])

            # ---------- Phase 0+1: local projections (K, V, Q zippered) ----
            with (
                tc.tile_pool(name="w0", bufs=1) as w_pool,
                tc.tile_pool(name="xc", bufs=3) as x_pool,
                tc.tile_pool(name="xq", bufs=3) as xq_pool,
                tc.tile_pool(name="vb", bufs=3) as vb_pool,
                tc.tile_pool(name="ps0", bufs=4, space="PSUM") as ps_pool,
            ):
                wk = w_pool.tile([P, 8, D], f16, tag="wk")
                wv = w_pool.tile([P, 8, D], f16, tag="wv")
                wq = w_pool.tile([P, 8, D], f16, tag="wq")
                for half in range(2):
                    nc.sync.dma_start(
                        out=wk[:, :, ds(half * CH, CH)],
                        in_=wkT[:, :, ds(half * CH, CH)],
                    )

                def load_xq(c):
                    xq = xq_pool.tile([P, 8, CH], f16, tag="xq", name=f"xq{c}")
                    nc.scalar.dma_start(
                        out=xq[:],
                        in_=xqT[c],
                    )
                    return xq

                xq_pending = []

                def proj_q(slot):
                    xq = xq_pending[slot]
                    for o in range(8):
                        ps = ps_pool.tile([P, CH], f32, tag="ps", name="psq")
                        for d in range(8):
                            nc.tensor.matmul(
                                ps[:],
                                lhsT=wq[:, d, ts(o, P)],
                                rhs=xq[:, d, :],
                                start=(d == 0),
                                stop=(d == 7),
                            )
                        nc.vector.tensor_copy(QTs[slot][:, o, :], ps[:])

                for c in range(8):
                    xc = x_pool.tile([P, 8, CH], f16, tag="xc", name=f"xc{c}")
                    nc.sync.dma_start(
                        out=xc[:],
                        in_=xT[c],
                    )
                    for o in range(8):
                        ps = ps_pool.tile([P, CH], f32, tag="ps", name="psk")
                        for d in range(8):
                            nc.tensor.matmul(
                                ps[:],
                                lhsT=wk[:, d, ts(o, P)],
                                rhs=xc[:, d, :],
                                start=(d == 0),
                                stop=(d == 7),
                            )
                        nc.vector.tensor_copy(
                            KTp[c // 2][:, o, ds((c % 2) * CH, CH)], ps[:]
                        )
                    if c == 0:
                        # deferred loads: SP/ACT reach these only after the
                        # first chunk's copies, leaving full DMA bandwidth to
                        # the critical wk+xc0 at kernel start
                        nc.sync.dma_start(
                            out=wv[:], in_=wvT[:]
                        )
                        nc.scalar.dma_start(
                            out=wq[:], in_=wqT[:]
                        )
                        xq_pending.append(load_xq(0))
                        xq_pending.append(load_xq(1))
                    for kt_i in range(4):
                        vb = vb_pool.tile([P, D], f16, tag="vb", name="vb")
                        for oh in range(2):
                            ps = ps_pool.tile([P, CH], f32, tag="ps", name="psv")
                            for d in range(8):
                                nc.tensor.matmul(
                                    ps[:],
                                    lhsT=xc[:, d, ts(kt_i, P)],
                                    rhs=wv[:, d, ts(oh, CH)],
                                    start=(d == 0),
                                    stop=(d == 7),
                                )
                            nc.scalar.copy(vb[:, ts(oh, CH)], ps[:])
                        nc.sync.dma_start(
                            out=vscr[ds(c * CH + kt_i * P, P), :], in_=vb[:]
                        )
                    if 1 <= c <= 4:
                        proj_q(c - 1)
                        if c <= 2:
                            xq_pending.append(load_xq(c + 1))

            # ---------------- Phase 2: attention ---------------------------
            with (
                tc.tile_pool(name="ctx", bufs=2) as ctx_pool,
                tc.tile_pool(name="vt", bufs=12) as v_pool,
                tc.tile_pool(name="pt", bufs=12) as p_pool,
                tc.tile_pool(name="et", bufs=3) as e_pool,
                tc.tile_pool(name="fo", bufs=3) as f_pool,
                tc.tile_pool(name="dsb", bufs=2) as den_pool,
                tc.tile_pool(name="pss", bufs=3, space="PSUM") as s_ps_pool,
                tc.tile_pool(name="psc", bufs=3, space="PSUM") as c_ps_pool,
                tc.tile_pool(name="psd", bufs=1, space="PSUM") as d_ps_pool,
                tc.tile_pool(name="psb", bufs=1, space="PSUM") as b_ps_pool,
            ):
                for slot in range(NSLOT):
                    nk = NK[slot]
                    ctx = ctx_pool.tile([P, 8, CH], f32, tag="ctx", name="ctx")
                    den = den_pool.tile([1, CH], f32, tag="den", name="den")
                    for blk in range(nk // 4):
                        p_tiles = []
                        v_tiles = []
                        for j4 in range(4):
                            j = blk * 4 + j4
                            vt = v_pool.tile([P, D], f16, tag="vt", name="vt")
                            nc.scalar.dma_start(out=vt[:], in_=vscr[ds(j * P, P), :])
                            sps = s_ps_pool.tile([P, CH], f32, name="sps")
                            for o in range(8):
                                nc.tensor.matmul(
                                    sps[:],
                                    lhsT=KTp[j // 8][:, o, ds((j % 8) * P, P)],
                                    rhs=QTs[slot][:, o, :],
                                    start=(o == 0),
                                    stop=(o == 7),
                                )
                            et = e_pool.tile([P, CH], f16, tag="et", name="et")
                            nc.scalar.activation(et[:], sps[:], Exp, scale=SCALE)
                            pt = p_pool.tile([P, CH], f16, tag="pt", name="pt")
                            col = SLOTBASE[slot] + j
                            nc.vector.scalar_tensor_tensor(
                                out=pt[:],
                                in0=dmat_sb[:],
                                scalar=amat_sb[:, ds(col, 1)],
                                in1=et[:],
                                op0=is_le,
                                op1=mult,
                            )
                            p_tiles.append(pt)
                            v_tiles.append(vt)
                        dps = d_ps_pool.tile([1, CH], f32, name="dps")
                        for j4 in range(4):
                            nc.tensor.matmul(
                                dps[:],
                                lhsT=ones_k_sb[:],
                                rhs=p_tiles[j4][:],
                                start=(j4 == 0),
                                stop=(j4 == 3),
                            )
                        if blk == 0:
                            nc.vector.tensor_copy(den[:], dps[:])
                        else:
                            nc.vector.tensor_add(den[:], den[:], dps[:])
                        for o in range(8):
                            cps = c_ps_pool.tile([P, CH], f32, name="cps")
                            for j4 in range(4):
                                nc.tensor.matmul(
                                    cps[:],
                                    lhsT=v_tiles[j4][:, ts(o, P)],
                                    rhs=p_tiles[j4][:],
                                    start=(j4 == 0),
                                    stop=(j4 == 3),
                                )
                            if blk == 0:
                                nc.vector.tensor_copy(ctx[:, o, :], cps[:])
                            else:
                                nc.vector.tensor_add(
                                    ctx[:, o, :], ctx[:, o, :], cps[:]
                                )
                    bps = b_ps_pool.tile([P, CH], f32, name="bps")
                    nc.tensor.matmul(
                        bps[:], lhsT=ones_r_sb[:], rhs=den[:], start=True, stop=True
                    )
                    rec = f_pool.tile([P, CH], f32, tag="rec", name="rec")
                    nc.vector.reciprocal(rec[:], bps[:])
                    for o in range(8):
                        ft = f_pool.tile([P, CH], f32, tag="ft", name="ft")
                        nc.vector.tensor_mul(ft[:], ctx[:, o, :], rec[:])
                        nc.sync.dma_start(
                            out=outT[ds(o * P, P), ts(slot, CH)], in_=ft[:]
                        )

    nc.compile()
    return nc


def _get_program():
    global _PROGRAM
    if _PROGRAM is None:
        _PROGRAM = _build_program()
    return _PROGRAM


def _make_in_maps(x, W_query, W_key, W_value):
    xT = np.ascontiguousarray(
        np.asarray(x, dtype=np.float32).transpose(0, 2, 1).astype(np.float16)
    )

    def tile_w(w):
        # [d, o] -> [p, d_slab, o]
        wt = np.asarray(w, dtype=np.float32).T.astype(np.float16)
        return np.ascontiguousarray(wt.reshape(8, P, D).transpose(1, 0, 2))

    def tile_x(xt, nch):
        # [d, s] -> [chunk, p, d_slab, s_off]
        return np.ascontiguousarray(
            xt.reshape(8, P, nch, CH).transpose(2, 1, 0, 3)
        )

    wqT = tile_w(W_query)
    wkT = tile_w(W_key)
    wvT = tile_w(W_value)
    dmat = (
        np.arange(P, dtype=np.float32)[:, None] - np.arange(CH, dtype=np.float32)[None, :]
    )
    dmat = np.ascontiguousarray(dmat.astype(np.float16))
    amat_h = []
    for h in range(2):
        a = np.zeros((P, 80), np.float16)
        for slot in range(NSLOT):
            cid = CHUNKS_H[h][slot]
            for j in range(NK[slot]):
                a[:, SLOTBASE[slot] + j] = CH * cid - P * j
        amat_h.append(a)
    ones_k = np.ones((P, 1), np.float16)
    ones_r = np.ones((1, P), np.float32)

    in_maps = []
    for core in range(8):
        b, h = core // 2, core % 2
        xq_cols = np.concatenate(
            [np.arange(c * CH, (c + 1) * CH) for c in CHUNKS_H[h]]
        )
        xqT_b = tile_x(np.ascontiguousarray(xT[b][:, xq_cols]), NSLOT)
        in_maps.append(
            {
                "xT": tile_x(xT[b], 8),
                "xqT": xqT_b,
                "wqT": wqT,
                "wkT": wkT,
                "wvT": wvT,
                "amat": amat_h[h],
                "dmat": dmat,
                "ones_k": ones_k,
                "ones_r": ones_r,
            }
        )
    return in_maps


def _assemble(results):
    out = np.empty((B, S, D), np.float32)
    for core in range(8):
        b, h = core // 2, core % 2
        oT = np.asarray(results[core]["outT"])  # [D, NQ]
        for slot, c in enumerate(CHUNKS_H[h]):
            out[b, c * CH : (c + 1) * CH, :] = oT[:, slot * CH : (slot + 1) * CH].T
    return out


def run(inputs, trace=False, trace_cores=None):
    """Run the kernel; returns (output, BassKernelResults)."""
    from concourse.bass_utils import run_bass_kernel_spmd

    nc = _get_program()
    in_maps = _make_in_maps(
        inputs["x"], inputs["W_query"], inputs["W_key"], inputs["W_value"]
    )
    kw = {}
    if trace:
        kw = dict(trace=True, trace_cores=trace_cores, stitch_traces=False)
    res = run_bass_kernel_spmd(nc, in_maps, list(range(8)), **kw)
    return _assemble(res.results), res


def kernel(x, W_query, W_key, W_value):
    out, _ = run({"x": x, "W_query": W_query, "W_key": W_key, "W_value": W_value})
    return out



# revision 4
# speedup vs baseline: 1.3275x; 1.3275x over previous
"""Causal single-head attention (B=4, S=4096, D=1024) on 8 TRN2 NeuronCores.

Sharding: core = (batch b, half h).  Each core computes attention output for
2048 queries of one batch: query chunks {0,3,4,7} (h=0) or {1,2,5,6} (h=1) of
8x512, which balances causal work.  Each core projects K^T/V for its full
batch (Q projections zippered in between the chunks); K^T lives in SBUF as
four independently-gated fp8 tiles and V is streamed from a DRAM scratch.

All heavy matmuls run fp8e4m3 with perf_mode=DoubleRow (2 contraction slabs
per pass), except a small fp16 "island" covering keys 0..511 for slot-0
queries (chunks 0/1): early causal queries have peaked softmax, so fp8
quantization of scores/V would land directly on the output there.

  K^T/Q^T/V projections:  psum = sum_d2 WT[d2,:,:128].T @ x^T[d2,:,:]  (fp8 DR)
  scores^T[k,q]        :  psum = sum_o2 KT[o2,:,k128].T @ QT[o2,:,q512] (fp8 DR)
  P = exp(s*scale) * causal_mask   (mask = (iota_k - iota_q) <= a[slot,j])
  den[1,q]             :  DVE-accumulate P tiles, then ones[k,1].T @ acc
  ctx^T[o,q]           :  psum = sum_j2 V2[j2,:,o128].T @ P2[j2,:,q512] (fp8 DR)
  out = ctx^T * (1/den)  broadcast via ones[1,128].T @ den[1,q]
"""

import sys

for _p in ("/opt/trn_rl_repo",):
    if _p not in sys.path:
        sys.path.insert(0, _p)

import numpy as np

B, S, D = 4, 4096, 1024
P = 128
CH = 512                       # query chunk
NSLOT = 4                      # chunks per core
NQ = NSLOT * CH                # queries per core
NK = [8, 16, 24, 32]           # k-tiles per slot (uniform across cores)
SLOTBASE = [0, 8, 24, 48]      # amat column base per slot
CHUNKS_H = [[0, 3, 4, 7], [1, 2, 5, 6]]
SCALE = 1.0 / 32.0             # 1/sqrt(D)

_PROGRAM = None


def _build_program():
    import concourse.bass as bass
    import concourse.tile as tile
    import concourse.mybir as mybir
    from concourse import bacc
    from concourse.bass import ds, ts

    f32 = mybir.dt.float32
    f16 = mybir.dt.float16
    f8 = mybir.dt.float8e4
    DR = mybir.MatmulPerfMode.DoubleRow

    nc = bacc.Bacc(trn_type="TRN2", target_bir_lowering=False, debug=False,
                   num_devices=8)

    xT8 = nc.declare_dram_parameter("xT8", [8, P, 8, CH], f8, isOutput=False)
    xc16d = nc.declare_dram_parameter("xc16", [P, 8, CH], f16, isOutput=False)
    xq8d = nc.declare_dram_parameter("xq8", [3, P, 8, CH], f8, isOutput=False)
    xq16d = nc.declare_dram_parameter("xq16", [P, 8, CH], f16, isOutput=False)
    wq8d = nc.declare_dram_parameter("wq8", [P, 8, D], f8, isOutput=False)
    wk8d = nc.declare_dram_parameter("wk8", [P, 8, D], f8, isOutput=False)
    wv8d = nc.declare_dram_parameter("wv8", [P, 8, D], f8, isOutput=False)
    wq16d = nc.declare_dram_parameter("wq16", [P, 8, D], f16, isOutput=False)
    wk16d = nc.declare_dram_parameter("wk16", [P, 8, D], f16, isOutput=False)
    wv16d = nc.declare_dram_parameter("wv16", [P, 8, D], f16, isOutput=False)
    amat = nc.declare_dram_parameter("amat", [P, 80], f16, isOutput=False)
    dmat = nc.declare_dram_parameter("dmat", [P, CH], f16, isOutput=False)
    ones_k = nc.declare_dram_parameter("ones_k", [P, 1], f16, isOutput=False)
    ones_r = nc.declare_dram_parameter("ones_r", [1, P], f16, isOutput=False)
    outT = nc.declare_dram_parameter("outT", [D, NQ], f16, isOutput=True)

    H = S // 4  # 1024: columns per resident K^T piece
    vscr8 = nc.dram_tensor("v_scratch8", [16, P, 2, D], f8)
    vscr16 = nc.dram_tensor("v_scratch16", [4, P, D], f16)

    Exp = mybir.ActivationFunctionType.Exp
    is_le = mybir.AluOpType.is_le
    mult = mybir.AluOpType.mult

    with tile.TileContext(nc, pool_alloc_mode="queue") as tc:
        with (
            tc.tile_pool(name="kt", bufs=1) as kt_pool,
            tc.tile_pool(name="qt", bufs=1) as qt_pool,
            tc.tile_pool(name="const", bufs=1) as const_pool,
        ):
            KTp = [
                kt_pool.tile([P, 8, H], f8, tag=f"kt{i}", name=f"KTp{i}")
                for i in range(4)
            ]
            KT16 = kt_pool.tile([P, 8, CH], f16, tag="kt16", name="KT16")
            QTs = [
                qt_pool.tile([P, 8, CH], f8, tag=f"qt{i}", name=f"QTs{i}")
                for i in range(NSLOT)
            ]
            QT16 = qt_pool.tile([P, 8, CH], f16, tag="qt16", name="QT16")
            dmat_sb = const_pool.tile([P, CH], f16, tag="dmat")
            amat_sb = const_pool.tile([P, 80], f16, tag="amat")
            ones_k_sb = const_pool.tile([P, 1], f16, tag="onesk")
            ones_r_sb = const_pool.tile([1, P], f16, tag="onesr")
            nc.sync.dma_start(out=dmat_sb[:], in_=dmat[:])
            nc.sync.dma_start(out=amat_sb[:], in_=amat[:])
            nc.sync.dma_start(out=ones_k_sb[:], in_=ones_k[:])
            nc.sync.dma_start(out=ones_r_sb[:], in_=ones_r[:])

            # ---------- Phase 0+1: local projections (K, V, Q zippered) ----
            with (
                tc.tile_pool(name="w0", bufs=1) as w_pool,
                tc.tile_pool(name="xc", bufs=2) as x_pool,
                tc.tile_pool(name="xq", bufs=2) as xq_pool,
                tc.tile_pool(name="vb", bufs=3) as vb_pool,
                tc.tile_pool(name="vb6", bufs=2) as vb16_pool,
                tc.tile_pool(name="ps0", bufs=4, space="PSUM") as ps_pool,
            ):
                wk8 = w_pool.tile([P, 8, D], f8, tag="wk8")
                wv8 = w_pool.tile([P, 8, D], f8, tag="wv8")
                wq8 = w_pool.tile([P, 8, D], f8, tag="wq8")
                wk16 = w_pool.tile([P, 8, D], f16, tag="wk16")
                wv16 = w_pool.tile([P, 8, D], f16, tag="wv16")
                wq16 = w_pool.tile([P, 8, D], f16, tag="wq16")
                xc16 = w_pool.tile([P, 8, CH], f16, tag="xc16")
                xq16 = w_pool.tile([P, 8, CH], f16, tag="xq16")
                # striped initial loads: the first fp8 chunk's weights
                for d2 in range(4):
                    nc.sync.dma_start(
                        out=wk8[:, ds(2 * d2, 2), :],
                        in_=wk8d[:, ds(2 * d2, 2), :],
                    )

                def load_xq(s):
                    # slot s in 1..3 (fp8)
                    xq = xq_pool.tile([P, 8, CH], f8, tag="xq", name=f"xq{s}")
                    nc.scalar.dma_start(out=xq[:], in_=xq8d[s - 1])
                    return xq

                xq_pending = {}

                def proj_q8(s):
                    xq = xq_pending[s]
                    for o in range(8):
                        ps = ps_pool.tile([P, CH], f32, tag="ps", name="psq")
                        for d2 in range(4):
                            nc.tensor.matmul(
                                ps[:],
                                lhsT=wq8[:, ds(2 * d2, 2), ts(o, P)],
                                rhs=xq[:, ds(2 * d2, 2), :],
                                start=(d2 == 0),
                                stop=(d2 == 3),
                                perf_mode=DR,
                            )
                        nc.vector.tensor_copy(QTs[s][:, o, :], ps[:])

                def proj_q16():
                    for o in range(8):
                        ps = ps_pool.tile([P, CH], f32, tag="ps", name="psq6")
                        for d in range(8):
                            nc.tensor.matmul(
                                ps[:],
                                lhsT=wq16[:, d, ts(o, P)],
                                rhs=xq16[:, d, :],
                                start=(d == 0),
                                stop=(d == 7),
                            )
                        nc.vector.tensor_copy(QT16[:, o, :], ps[:])
                        nc.scalar.copy(QTs[0][:, o, :], ps[:])

                def proj_kv16():
                    # chunk 0 in fp16, dual-cast fp8 copies
                    for o in range(8):
                        ps = ps_pool.tile([P, CH], f32, tag="ps", name="psk6")
                        for d in range(8):
                            nc.tensor.matmul(
                                ps[:],
                                lhsT=wk16[:, d, ts(o, P)],
                                rhs=xc16[:, d, :],
                                start=(d == 0),
                                stop=(d == 7),
                            )
                        nc.vector.tensor_copy(KT16[:, o, :], ps[:])
                        nc.scalar.copy(KTp[0][:, o, ds(0, CH)], ps[:])
                    for kt_i in range(4):
                        vb16 = vb16_pool.tile([P, D], f16, tag="vb16", name="vb16")
                        vb8 = vb_pool.tile([P, D], f8, tag="vb", name="vb8c0")
                        for oh in range(2):
                            ps = ps_pool.tile([P, CH], f32, tag="ps", name="psv6")
                            for d in range(8):
                                nc.tensor.matmul(
                                    ps[:],
                                    lhsT=xc16[:, d, ts(kt_i, P)],
                                    rhs=wv16[:, d, ts(oh, CH)],
                                    start=(d == 0),
                                    stop=(d == 7),
                                )
                            nc.scalar.copy(vb16[:, ts(oh, CH)], ps[:])
                            nc.vector.tensor_copy(vb8[:, ts(oh, CH)], ps[:])
                        nc.sync.dma_start(out=vscr16[kt_i], in_=vb16[:])
                        nc.sync.dma_start(
                            out=vscr8[kt_i // 2][:, kt_i % 2, :], in_=vb8[:]
                        )

                # chunk processing order: fp8 chunk 1 first (cheap startup
                # DMA), then the fp16 island chunk 0 while its bigger fp16
                # weights stream in, then the rest.
                c_order = [1, 0, 2, 3, 4, 5, 6, 7]
                # Q slots zippered after iterations 1..4 (slot 0 = fp16 last)
                q_sched = {1: 1, 2: 2, 3: 3, 4: 0}
                for it, c in enumerate(c_order):
                    if c == 0:
                        proj_kv16()
                    else:
                        xc = x_pool.tile([P, 8, CH], f8, tag="xc", name=f"xc{c}")
                        nsplit = 4 if it == 0 else 2
                        for sp in range(nsplit):
                            w = 8 // nsplit
                            nc.sync.dma_start(
                                out=xc[:, ds(sp * w, w), :],
                                in_=xT8[c][:, ds(sp * w, w), :],
                            )
                        for o in range(8):
                            ps = ps_pool.tile([P, CH], f32, tag="ps", name="psk")
                            for d2 in range(4):
                                nc.tensor.matmul(
                                    ps[:],
                                    lhsT=wk8[:, ds(2 * d2, 2), ts(o, P)],
                                    rhs=xc[:, ds(2 * d2, 2), :],
                                    start=(d2 == 0),
                                    stop=(d2 == 3),
                                    perf_mode=DR,
                                )
                            nc.vector.tensor_copy(
                                KTp[c // 2][:, o, ds((c % 2) * CH, CH)], ps[:]
                            )
                        if it == 0:
                            # deferred loads, enqueued between this chunk's
                            # K and V work: wv8 must precede the V-proj
                            # copies in the ACT queue (else the stalled
                            # copies would block the wv8 trigger), and the
                            # critical wk8+xc loads above go first on sync.
                            for d2 in range(4):
                                nc.scalar.dma_start(
                                    out=wv8[:, ds(2 * d2, 2), :],
                                    in_=wv8d[:, ds(2 * d2, 2), :],
                                )
                            nc.scalar.dma_start(out=wq8[:], in_=wq8d[:])
                            for sp in range(4):
                                nc.sync.dma_start(
                                    out=wk16[:, ds(2 * sp, 2), :],
                                    in_=wk16d[:, ds(2 * sp, 2), :],
                                )
                            nc.sync.dma_start(out=xc16[:], in_=xc16d[:])
                            for sp in range(4):
                                nc.sync.dma_start(
                                    out=wv16[:, ds(2 * sp, 2), :],
                                    in_=wv16d[:, ds(2 * sp, 2), :],
                                )
                            nc.scalar.dma_start(out=wq16[:], in_=wq16d[:])
                            nc.scalar.dma_start(out=xq16[:], in_=xq16d[:])
                            xq_pending[1] = load_xq(1)
                            xq_pending[2] = load_xq(2)
                        for kt_i in range(4):
                            vb = vb_pool.tile([P, D], f8, tag="vb", name="vb")
                            for oh in range(2):
                                ps = ps_pool.tile([P, CH], f32, tag="ps", name="psv")
                                for d2 in range(4):
                                    nc.tensor.matmul(
                                        ps[:],
                                        lhsT=xc[:, ds(2 * d2, 2), ts(kt_i, P)],
                                        rhs=wv8[:, ds(2 * d2, 2), ts(oh, CH)],
                                        start=(d2 == 0),
                                        stop=(d2 == 3),
                                        perf_mode=DR,
                                    )
                                nc.scalar.copy(vb[:, ts(oh, CH)], ps[:])
                            j = c * 4 + kt_i
                            nc.sync.dma_start(
                                out=vscr8[j // 2][:, j % 2, :], in_=vb[:]
                            )
                    sq = q_sched.get(it)
                    if sq is not None:
                        if sq == 0:
                            proj_q16()
                        else:
                            proj_q8(sq)
                        if sq == 2:
                            xq_pending[3] = load_xq(3)

            # ---------------- Phase 2: attention ---------------------------
            with (
                tc.tile_pool(name="ctx", bufs=2) as ctx_pool,
                tc.tile_pool(name="vt", bufs=8) as v_pool,
                tc.tile_pool(name="v6", bufs=4) as v16_pool,
                tc.tile_pool(name="pt", bufs=8) as p_pool,
                tc.tile_pool(name="p6", bufs=4) as p16_pool,
                tc.tile_pool(name="et", bufs=3) as e_pool,
                tc.tile_pool(name="fo", bufs=4) as f_pool,
                tc.tile_pool(name="dsb", bufs=2) as den_pool,
                tc.tile_pool(name="pss", bufs=3, space="PSUM") as s_ps_pool,
                tc.tile_pool(name="psc", bufs=3, space="PSUM") as c_ps_pool,
                tc.tile_pool(name="psd", bufs=1, space="PSUM") as d_ps_pool,
                tc.tile_pool(name="psb", bufs=1, space="PSUM") as b_ps_pool,
            ):
                for slot in range(NSLOT):
                    nk = NK[slot]
                    ctx = ctx_pool.tile([P, 8, CH], f32, tag="ctx", name="ctx")
                    acc = den_pool.tile([P, CH], f16, tag="acc", name="acc")
                    for blk in range(nk // 8):
                        pairs = []
                        island_tiles = []
                        cur_vt2 = cur_pt2 = None
                        for jj in range(8):
                            j = blk * 8 + jj
                            island = slot == 0 and blk == 0 and jj < 4
                            sps = s_ps_pool.tile([P, CH], f32, name="sps")
                            if island:
                                for o in range(8):
                                    nc.tensor.matmul(
                                        sps[:],
                                        lhsT=KT16[:, o, ds(j * P, P)],
                                        rhs=QT16[:, o, :],
                                        start=(o == 0),
                                        stop=(o == 7),
                                    )
                            else:
                                for o2 in range(4):
                                    nc.tensor.matmul(
                                        sps[:],
                                        lhsT=KTp[j // 8][
                                            :, ds(2 * o2, 2), ds((j % 8) * P, P)
                                        ],
                                        rhs=QTs[slot][:, ds(2 * o2, 2), :],
                                        start=(o2 == 0),
                                        stop=(o2 == 3),
                                        perf_mode=DR,
                                    )
                            et = e_pool.tile([P, CH], f16, tag="et", name="et")
                            nc.scalar.activation(et[:], sps[:], Exp, scale=SCALE)
                            col = SLOTBASE[slot] + j
                            if island:
                                pt = p16_pool.tile(
                                    [P, CH], f16, tag="pt16", name="pt16"
                                )
                                nc.vector.scalar_tensor_tensor(
                                    out=pt[:],
                                    in0=dmat_sb[:],
                                    scalar=amat_sb[:, ds(col, 1)],
                                    in1=et[:],
                                    op0=is_le,
                                    op1=mult,
                                )
                                vt = v16_pool.tile(
                                    [P, D], f16, tag="vt16", name="vt16"
                                )
                                nc.gpsimd.dma_start(out=vt[:], in_=vscr16[j])
                                island_tiles.append((vt, pt))
                                ptview = pt[:]
                            else:
                                if jj % 2 == 0:
                                    cur_pt2 = p_pool.tile(
                                        [P, 2, CH], f8, tag="pt", name="pt2"
                                    )
                                    cur_vt2 = v_pool.tile(
                                        [P, 2, D], f8, tag="vt", name="vt2"
                                    )
                                    nc.gpsimd.dma_start(
                                        out=cur_vt2[:], in_=vscr8[j // 2]
                                    )
                                nc.vector.scalar_tensor_tensor(
                                    out=cur_pt2[:, jj % 2, :],
                                    in0=dmat_sb[:],
                                    scalar=amat_sb[:, ds(col, 1)],
                                    in1=et[:],
                                    op0=is_le,
                                    op1=mult,
                                )
                                ptview = cur_pt2[:, jj % 2, :]
                                if jj % 2 == 1:
                                    pairs.append((cur_vt2, cur_pt2))
                            if blk == 0 and jj == 0:
                                nc.vector.tensor_copy(acc[:], ptview)
                            else:
                                nc.vector.tensor_add(acc[:], acc[:], ptview)
                        n_mm = len(island_tiles) + len(pairs)
                        for o in range(8):
                            cps = c_ps_pool.tile([P, CH], f32, name="cps")
                            idx = 0
                            for vt, pt in island_tiles:
                                nc.tensor.matmul(
                                    cps[:],
                                    lhsT=vt[:, ts(o, P)],
                                    rhs=pt[:],
                                    start=(idx == 0),
                                    stop=(idx == n_mm - 1),
                                )
                                idx += 1
                            for vt2, pt2 in pairs:
                                nc.tensor.matmul(
                                    cps[:],
                                    lhsT=vt2[:, :, ts(o, P)],
                                    rhs=pt2[:],
                                    start=(idx == 0),
                                    stop=(idx == n_mm - 1),
                                    perf_mode=DR,
                                )
                                idx += 1
                            if blk == 0:
                                nc.vector.tensor_copy(ctx[:, o, :], cps[:])
                            else:
                                nc.vector.tensor_add(
                                    ctx[:, o, :], ctx[:, o, :], cps[:]
                                )
                    dps = d_ps_pool.tile([1, CH], f32, name="dps")
                    nc.tensor.matmul(
                        dps[:], lhsT=ones_k_sb[:], rhs=acc[:], start=True, stop=True
                    )
                    den_sb = f_pool.tile([1, CH], f16, tag="den", name="den")
                    nc.vector.tensor_copy(den_sb[:], dps[:])
                    bps = b_ps_pool.tile([P, CH], f32, name="bps")
                    nc.tensor.matmul(
                        bps[:], lhsT=ones_r_sb[:], rhs=den_sb[:], start=True,
                        stop=True,
                    )
                    rec = f_pool.tile([P, CH], f32, tag="rec", name="rec")
                    nc.vector.reciprocal(rec[:], bps[:])
                    for o in range(8):
                        ft = f_pool.tile([P, CH], f16, tag="ft", name="ft")
                        nc.vector.tensor_mul(ft[:], ctx[:, o, :], rec[:])
                        nc.sync.dma_start(
                            out=outT[ds(o * P, P), ts(slot, CH)], in_=ft[:]
                        )

    nc.compile()
    return nc


def _get_program():
    global _PROGRAM
    if _PROGRAM is None:
        _PROGRAM = _build_program()
    return _PROGRAM


def _make_in_maps(x, W_query, W_key, W_value):
    import ml_dtypes

    f8 = ml_dtypes.float8_e4m3

    xT = np.ascontiguousarray(
        np.asarray(x, dtype=np.float32).transpose(0, 2, 1).astype(np.float16)
    )
    xT8 = xT.astype(f8)

    def tile_w(w, dt):
        # [d, o] -> [p, d_slab, o]
        wt = np.asarray(w, dtype=np.float32).T.astype(np.float16).astype(dt)
        return np.ascontiguousarray(wt.reshape(8, P, D).transpose(1, 0, 2))

    def tile_x(xt, nch):
        # [d, s] -> [chunk, p, d_slab, s_off]
        return np.ascontiguousarray(
            xt.reshape(8, P, nch, CH).transpose(2, 1, 0, 3)
        )

    w8 = {k: tile_w(w, f8) for k, w in
          (("wq8", W_query), ("wk8", W_key), ("wv8", W_value))}
    w16 = {k: tile_w(w, np.float16) for k, w in
           (("wq16", W_query), ("wk16", W_key), ("wv16", W_value))}
    dmat = (
        np.arange(P, dtype=np.float32)[:, None]
        - np.arange(CH, dtype=np.float32)[None, :]
    )
    dmat = np.ascontiguousarray(dmat.astype(np.float16))
    amat_h = []
    for h in range(2):
        a = np.zeros((P, 80), np.float16)
        for slot in range(NSLOT):
            cid = CHUNKS_H[h][slot]
            for j in range(NK[slot]):
                a[:, SLOTBASE[slot] + j] = CH * cid - P * j
        amat_h.append(a)
    ones_k = np.ones((P, 1), np.float16)
    ones_r = np.ones((1, P), np.float16)

    in_maps = []
    for core in range(8):
        b, h = core // 2, core % 2
        chunks = CHUNKS_H[h]
        xq8 = np.stack(
            [tile_x(xT8[b][:, c * CH : (c + 1) * CH], 1)[0] for c in chunks[1:]]
        )
        xq16 = tile_x(
            xT[b][:, chunks[0] * CH : (chunks[0] + 1) * CH], 1
        )[0]
        in_maps.append(
            {
                "xT8": tile_x(xT8[b], 8),
                "xc16": tile_x(xT[b][:, :CH], 1)[0],
                "xq8": xq8,
                "xq16": xq16,
                **w8,
                **w16,
                "amat": amat_h[h],
                "dmat": dmat,
                "ones_k": ones_k,
                "ones_r": ones_r,
            }
        )
    return in_maps


def _assemble(results):
    out = np.empty((B, S, D), np.float32)
    for core in range(8):
        b, h = core // 2, core % 2
        oT = np.asarray(results[core]["outT"]).astype(np.float32)  # [D, NQ]
        for slot, c in enumerate(CHUNKS_H[h]):
            out[b, c * CH : (c + 1) * CH, :] = oT[:, slot * CH : (slot + 1) * CH].T
    return out


def run(inputs, trace=False, trace_cores=None):
    """Run the kernel; returns (output, BassKernelResults)."""
    from concourse.bass_utils import run_bass_kernel_spmd

    nc = _get_program()
    in_maps = _make_in_maps(
        inputs["x"], inputs["W_query"], inputs["W_key"], inputs["W_value"]
    )
    kw = {}
    if trace:
        kw = dict(trace=True, trace_cores=trace_cores, stitch_traces=False)
    res = run_bass_kernel_spmd(nc, in_maps, list(range(8)), **kw)
    return _assemble(res.results), res


def kernel(x, W_query, W_key, W_value):
    out, _ = run({"x": x, "W_query": W_query, "W_key": W_key, "W_value": W_value})
    return out


# revision 8
# speedup vs baseline: 1.7822x; 1.3425x over previous
"""Causal single-head attention (B=4, S=4096, D=1024) on 8 TRN2 NeuronCores.

Sharding: core = (batch b, half h).  Each core computes attention output for
2048 queries of one batch: query chunks {0,3,4,7} (h=0) or {1,2,5,6} (h=1) of
8x512, which balances causal work.  Each core projects K^T/V for its full
batch (Q projections zippered in between the chunks); K^T lives in SBUF as
four independently-gated fp8 tiles and V is streamed from a DRAM scratch.

All heavy matmuls run fp8e4m3 with perf_mode=DoubleRow (2 contraction slabs
per pass), except a small fp16 "island" covering keys 0..511 for slot-0
queries (chunks 0/1): early causal queries have peaked softmax, so fp8
quantization of scores/V would land directly on the output there.

  K^T/Q^T/V projections:  psum = sum_d2 WT[d2,:,:128].T @ x^T[d2,:,:]  (fp8 DR)
  scores^T[k,q]        :  psum = sum_o2 KT[o2,:,k128].T @ QT[o2,:,q512] (fp8 DR)
  P = exp(s*scale) * causal_mask   (mask = (iota_k - iota_q) <= a[slot,j])
  den[1,q]             :  DVE-accumulate P tiles, then ones[k,1].T @ acc
  ctx^T[o,q]           :  psum = sum_j2 V2[j2,:,o128].T @ P2[j2,:,q512] (fp8 DR)
  out = ctx^T * (1/den)  broadcast via ones[1,128].T @ den[1,q]
"""

import sys

for _p in ("/opt/trn_rl_repo",):
    if _p not in sys.path:
        sys.path.insert(0, _p)

import numpy as np

B, S, D = 4, 4096, 1024
P = 128
CH = 512                       # query chunk
NSLOT = 4                      # chunks per core
NQ = NSLOT * CH                # queries per core
NK = [8, 16, 24, 32]           # k-tiles per slot (uniform across cores)
SLOTBASE = [0, 8, 24, 48]      # amat column base per slot
CHUNKS_H = [[0, 3, 4, 7], [1, 2, 5, 6]]
SCALE = 1.0 / 32.0             # 1/sqrt(D)

_PROGRAM = None


def _build_program():
    import concourse.bass as bass
    import concourse.tile as tile
    import concourse.mybir as mybir
    from concourse import bacc
    from concourse.bass import ds, ts

    f32 = mybir.dt.float32
    f16 = mybir.dt.float16
    f8 = mybir.dt.float8e4
    DR = mybir.MatmulPerfMode.DoubleRow

    nc = bacc.Bacc(trn_type="TRN2", target_bir_lowering=False, debug=False,
                   num_devices=8)

    xT8 = nc.declare_dram_parameter("xT8", [8, P, 8, CH], f8, isOutput=False)
    xc16d = nc.declare_dram_parameter("xc16", [P, 8, CH], f16, isOutput=False)
    xq8d = nc.declare_dram_parameter("xq8", [3, P, 8, CH], f8, isOutput=False)
    xq16d = nc.declare_dram_parameter("xq16", [P, 8, CH], f16, isOutput=False)
    wq8d = nc.declare_dram_parameter("wq8", [P, 8, D], f8, isOutput=False)
    wk8d = nc.declare_dram_parameter("wk8", [P, 8, D], f8, isOutput=False)
    wv8d = nc.declare_dram_parameter("wv8", [P, 8, D], f8, isOutput=False)
    wq16d = nc.declare_dram_parameter("wq16", [P, 8, D], f16, isOutput=False)
    wk16d = nc.declare_dram_parameter("wk16", [P, 8, D], f16, isOutput=False)
    wv16d = nc.declare_dram_parameter("wv16", [P, 8, D], f16, isOutput=False)
    amat = nc.declare_dram_parameter("amat", [P, 80], f16, isOutput=False)
    dmat = nc.declare_dram_parameter("dmat", [P, CH], f16, isOutput=False)
    ones_k = nc.declare_dram_parameter("ones_k", [P, 1], f16, isOutput=False)
    ones_r = nc.declare_dram_parameter("ones_r", [1, P], f16, isOutput=False)
    outT = nc.declare_dram_parameter("outT", [D, NQ], f16, isOutput=True)

    H = S // 4  # 1024: columns per resident K^T piece
    vscr8 = nc.dram_tensor("v_scratch8", [16, P, 2, D], f8)
    vscr16 = nc.dram_tensor("v_scratch16", [4, P, D], f16)

    Exp = mybir.ActivationFunctionType.Exp
    is_le = mybir.AluOpType.is_le
    mult = mybir.AluOpType.mult

    with tile.TileContext(nc, pool_alloc_mode="queue") as tc:
        with (
            tc.tile_pool(name="kt", bufs=1) as kt_pool,
            tc.tile_pool(name="qt", bufs=1) as qt_pool,
            tc.tile_pool(name="const", bufs=1) as const_pool,
        ):
            KTp = [
                kt_pool.tile([P, 8, H], f8, tag=f"kt{i}", name=f"KTp{i}")
                for i in range(4)
            ]
            KT16 = kt_pool.tile([P, 8, CH], f16, tag="kt16", name="KT16")
            QTs = [
                qt_pool.tile([P, 8, CH], f8, tag=f"qt{i}", name=f"QTs{i}")
                for i in range(NSLOT)
            ]
            QT16 = qt_pool.tile([P, 8, CH], f16, tag="qt16", name="QT16")
            dmat_sb = const_pool.tile([P, CH], f16, tag="dmat")
            amat_sb = const_pool.tile([P, 80], f16, tag="amat")
            ones_k_sb = const_pool.tile([P, 1], f16, tag="onesk")
            ones_r_sb = const_pool.tile([1, P], f16, tag="onesr")
            nc.gpsimd.dma_start(out=dmat_sb[:], in_=dmat[:])
            nc.gpsimd.dma_start(out=amat_sb[:], in_=amat[:])
            nc.gpsimd.dma_start(out=ones_k_sb[:], in_=ones_k[:])
            nc.gpsimd.dma_start(out=ones_r_sb[:], in_=ones_r[:])

            # ---------- Phase 0+1: local projections (K, V, Q zippered) ----
            with (
                tc.tile_pool(name="w0", bufs=1) as w_pool,
                tc.tile_pool(name="xc", bufs=2) as x_pool,
                tc.tile_pool(name="xq", bufs=2) as xq_pool,
                tc.tile_pool(name="vb", bufs=3) as vb_pool,
                tc.tile_pool(name="vb6", bufs=2) as vb16_pool,
                tc.tile_pool(name="ps0", bufs=4, space="PSUM") as ps_pool,
            ):
                wk8 = w_pool.tile([P, 8, D], f8, tag="wk8")
                wv8 = w_pool.tile([P, 8, D], f8, tag="wv8")
                wq8 = w_pool.tile([P, 8, D], f8, tag="wq8")
                wk16 = w_pool.tile([P, 8, D], f16, tag="wk16")
                wv16 = w_pool.tile([P, 8, D], f16, tag="wv16")
                wq16 = w_pool.tile([P, 8, D], f16, tag="wq16")
                xc16 = w_pool.tile([P, 8, CH], f16, tag="xc16")
                xq16 = w_pool.tile([P, 8, CH], f16, tag="xq16")
                # striped initial loads: the first fp8 chunk's weights
                for d2 in range(4):
                    eng = nc.sync if d2 < 2 else nc.scalar
                    eng.dma_start(
                        out=wk8[:, ds(2 * d2, 2), :],
                        in_=wk8d[:, ds(2 * d2, 2), :],
                    )

                def load_xq(s):
                    # slot s in 1..3 (fp8)
                    xq = xq_pool.tile([P, 8, CH], f8, tag="xq", name=f"xq{s}")
                    nc.scalar.dma_start(out=xq[:], in_=xq8d[s - 1])
                    return xq

                xq_pending = {}

                def proj_q8(s):
                    xq = xq_pending[s]
                    for o in range(8):
                        ps = ps_pool.tile([P, CH], f32, tag="ps", name="psq")
                        for d2 in range(4):
                            nc.tensor.matmul(
                                ps[:],
                                lhsT=wq8[:, ds(2 * d2, 2), ts(o, P)],
                                rhs=xq[:, ds(2 * d2, 2), :],
                                start=(d2 == 0),
                                stop=(d2 == 3),
                                perf_mode=DR,
                            )
                        nc.vector.tensor_copy(QTs[s][:, o, :], ps[:])

                def proj_q16():
                    for o in range(8):
                        ps = ps_pool.tile([P, CH], f32, tag="ps", name="psq6")
                        for d in range(8):
                            nc.tensor.matmul(
                                ps[:],
                                lhsT=wq16[:, d, ts(o, P)],
                                rhs=xq16[:, d, :],
                                start=(d == 0),
                                stop=(d == 7),
                            )
                        nc.vector.tensor_copy(QT16[:, o, :], ps[:])
                        nc.scalar.copy(QTs[0][:, o, :], ps[:])

                def proj_kv16():
                    # chunk 0 in fp16, dual-cast fp8 copies
                    for o in range(8):
                        ps = ps_pool.tile([P, CH], f32, tag="ps", name="psk6")
                        for d in range(8):
                            nc.tensor.matmul(
                                ps[:],
                                lhsT=wk16[:, d, ts(o, P)],
                                rhs=xc16[:, d, :],
                                start=(d == 0),
                                stop=(d == 7),
                            )
                        nc.vector.tensor_copy(KT16[:, o, :], ps[:])
                        nc.scalar.copy(KTp[0][:, o, ds(0, CH)], ps[:])
                    for kt_i in range(4):
                        vb16 = vb16_pool.tile([P, D], f16, tag="vb16", name="vb16")
                        vb8 = vb_pool.tile([P, D], f8, tag="vb", name="vb8c0")
                        for oh in range(2):
                            ps = ps_pool.tile([P, CH], f32, tag="ps", name="psv6")
                            for d in range(8):
                                nc.tensor.matmul(
                                    ps[:],
                                    lhsT=xc16[:, d, ts(kt_i, P)],
                                    rhs=wv16[:, d, ts(oh, CH)],
                                    start=(d == 0),
                                    stop=(d == 7),
                                )
                            nc.scalar.copy(vb16[:, ts(oh, CH)], ps[:])
                            nc.vector.tensor_copy(vb8[:, ts(oh, CH)], ps[:])
                        nc.sync.dma_start(out=vscr16[kt_i], in_=vb16[:])
                        nc.sync.dma_start(
                            out=vscr8[kt_i // 2][:, kt_i % 2, :], in_=vb8[:]
                        )

                # chunk processing order: fp8 chunk 1 first (cheap startup
                # DMA), then the fp16 island chunk 0 while its bigger fp16
                # weights stream in, then the rest.
                c_order = [1, 0, 2, 3, 4, 5, 6, 7]
                # Q slots zippered after iterations 1..4 (slot 0 = fp16 last)
                q_sched = {1: 1, 2: 2, 3: 3, 4: 0}
                for it, c in enumerate(c_order):
                    if c == 0:
                        proj_kv16()
                    else:
                        xc = x_pool.tile([P, 8, CH], f8, tag="xc", name=f"xc{c}")
                        nsplit = 4 if it == 0 else 2
                        for sp in range(nsplit):
                            w = 8 // nsplit
                            if nsplit == 2:
                                eng = nc.sync
                            else:
                                eng = nc.gpsimd if sp % 2 == 0 else nc.sync
                            eng.dma_start(
                                out=xc[:, ds(sp * w, w), :],
                                in_=xT8[c][:, ds(sp * w, w), :],
                            )
                        for o in range(8):
                            ps = ps_pool.tile([P, CH], f32, tag="ps", name="psk")
                            for d2 in range(4):
                                nc.tensor.matmul(
                                    ps[:],
                                    lhsT=wk8[:, ds(2 * d2, 2), ts(o, P)],
                                    rhs=xc[:, ds(2 * d2, 2), :],
                                    start=(d2 == 0),
                                    stop=(d2 == 3),
                                    perf_mode=DR,
                                )
                            if o % 2 == 0:
                                nc.vector.tensor_copy(
                                    KTp[c // 2][:, o, ds((c % 2) * CH, CH)],
                                    ps[:],
                                )
                            else:
                                nc.scalar.copy(
                                    KTp[c // 2][:, o, ds((c % 2) * CH, CH)],
                                    ps[:],
                                )
                        if it == 0:
                            # deferred loads, enqueued between this chunk's
                            # K and V work: wv8 must precede the V-proj
                            # copies in the ACT queue (else the stalled
                            # copies would block the wv8 trigger), and the
                            # critical wk8+xc loads above go first on sync.
                            for d2 in range(4):
                                nc.scalar.dma_start(
                                    out=wv8[:, ds(2 * d2, 2), :],
                                    in_=wv8d[:, ds(2 * d2, 2), :],
                                )
                            nc.scalar.dma_start(out=wq8[:], in_=wq8d[:])
                            for sp in range(4):
                                nc.sync.dma_start(
                                    out=wk16[:, ds(2 * sp, 2), :],
                                    in_=wk16d[:, ds(2 * sp, 2), :],
                                )
                            nc.sync.dma_start(out=xc16[:], in_=xc16d[:])
                            for sp in range(4):
                                nc.sync.dma_start(
                                    out=wv16[:, ds(2 * sp, 2), :],
                                    in_=wv16d[:, ds(2 * sp, 2), :],
                                )
                            nc.scalar.dma_start(out=wq16[:], in_=wq16d[:])
                            nc.scalar.dma_start(out=xq16[:], in_=xq16d[:])
                            xq_pending[1] = load_xq(1)
                            xq_pending[2] = load_xq(2)
                        for kt_i in range(4):
                            vb = vb_pool.tile([P, D], f8, tag="vb", name="vb")
                            for oh in range(2):
                                ps = ps_pool.tile([P, CH], f32, tag="ps", name="psv")
                                for d2 in range(4):
                                    nc.tensor.matmul(
                                        ps[:],
                                        lhsT=xc[:, ds(2 * d2, 2), ts(kt_i, P)],
                                        rhs=wv8[:, ds(2 * d2, 2), ts(oh, CH)],
                                        start=(d2 == 0),
                                        stop=(d2 == 3),
                                        perf_mode=DR,
                                    )
                                nc.scalar.copy(vb[:, ts(oh, CH)], ps[:])
                            j = c * 4 + kt_i
                            nc.sync.dma_start(
                                out=vscr8[j // 2][:, j % 2, :], in_=vb[:]
                            )
                    sq = q_sched.get(it)
                    if sq is not None:
                        if sq == 0:
                            proj_q16()
                        else:
                            proj_q8(sq)
                        if sq == 2:
                            xq_pending[3] = load_xq(3)

            # ---------------- Phase 2: attention ---------------------------
            # Per slot: all score k-tiles first (P tiles + den accumulate),
            # then ctx as ONE psum accumulation group per o (no SBUF ctx
            # accumulator, ft multiplies read the psum directly).
            # Tiles causally full on BOTH halves skip the mask STT: ACT
            # writes exp() straight to the fp8 P pair tile.
            CLEAN_NK = [
                min(4 * CHUNKS_H[0][s], 4 * CHUNKS_H[1][s], NK[s])
                for s in range(NSLOT)
            ]
            Ln = mybir.ActivationFunctionType.Ln
            with (
                tc.tile_pool(name="vt", bufs=28) as v_pool,
                tc.tile_pool(name="v6", bufs=4) as v16_pool,
                tc.tile_pool(name="pt", bufs=20) as p_pool,
                tc.tile_pool(name="p6", bufs=4) as p16_pool,
                tc.tile_pool(name="et", bufs=3) as e_pool,
                tc.tile_pool(name="fo", bufs=6) as f_pool,
                tc.tile_pool(name="dsb", bufs=2) as den_pool,
                tc.tile_pool(name="pss", bufs=3, space="PSUM") as s_ps_pool,
                tc.tile_pool(name="psc", bufs=3, space="PSUM") as c_ps_pool,
                tc.tile_pool(name="psd", bufs=1, space="PSUM") as d_ps_pool,
                tc.tile_pool(name="psb", bufs=1, space="PSUM") as b_ps_pool,
            ):
                DMA_ENGS = [nc.gpsimd, nc.scalar, nc.sync]

                def prefetch_v(slot):
                    tiles = {"i": [], "p": []}
                    qi = 0
                    if slot == 0:
                        for j in range(4):
                            vt = v16_pool.tile([P, D], f16, tag="vt16",
                                               name="vt16")
                            DMA_ENGS[qi % 3].dma_start(out=vt[:], in_=vscr16[j])
                            qi += 1
                            tiles["i"].append(vt)
                        prange = range(2, 4)
                    else:
                        prange = range(NK[slot] // 2)
                    for pr in prange:
                        vt2 = v_pool.tile([P, 2, D], f8, tag="vt", name="vt2")
                        DMA_ENGS[qi % 3].dma_start(out=vt2[:], in_=vscr8[pr])
                        qi += 1
                        tiles["p"].append(vt2)
                    return tiles

                vtiles = prefetch_v(0)
                for slot in range(NSLOT):
                    nk = NK[slot]
                    acc = den_pool.tile([P, CH], f16, tag="acc", name="acc")
                    pt16s = []
                    pt2s = []
                    cur_pt2 = None
                    for j in range(nk):
                        island = slot == 0 and j < 4
                        clean = j < CLEAN_NK[slot]
                        sps = s_ps_pool.tile([P, CH], f32, name="sps")
                        if island:
                            for o in range(8):
                                nc.tensor.matmul(
                                    sps[:],
                                    lhsT=KT16[:, o, ds(j * P, P)],
                                    rhs=QT16[:, o, :],
                                    start=(o == 0),
                                    stop=(o == 7),
                                )
                        else:
                            for o2 in range(4):
                                nc.tensor.matmul(
                                    sps[:],
                                    lhsT=KTp[j // 8][
                                        :, ds(2 * o2, 2), ds((j % 8) * P, P)
                                    ],
                                    rhs=QTs[slot][:, ds(2 * o2, 2), :],
                                    start=(o2 == 0),
                                    stop=(o2 == 3),
                                    perf_mode=DR,
                                )
                        if not island and j % 2 == 0:
                            cur_pt2 = p_pool.tile([P, 2, CH], f8, tag="pt",
                                                  name="pt2")
                        col = SLOTBASE[slot] + j
                        if island:
                            et = e_pool.tile([P, CH], f16, tag="et", name="et")
                            nc.scalar.activation(et[:], sps[:], Exp, scale=SCALE)
                            pt = p16_pool.tile([P, CH], f16, tag="pt16",
                                               name="pt16")
                            nc.vector.scalar_tensor_tensor(
                                out=pt[:],
                                in0=dmat_sb[:],
                                scalar=amat_sb[:, ds(col, 1)],
                                in1=et[:],
                                op0=is_le,
                                op1=mult,
                            )
                            pt16s.append(pt)
                            ptv = pt[:]
                        elif clean:
                            nc.scalar.activation(
                                cur_pt2[:, j % 2, :], sps[:], Exp, scale=SCALE
                            )
                            ptv = cur_pt2[:, j % 2, :]
                        else:
                            et = e_pool.tile([P, CH], f16, tag="et", name="et")
                            nc.scalar.activation(et[:], sps[:], Exp, scale=SCALE)
                            nc.vector.scalar_tensor_tensor(
                                out=cur_pt2[:, j % 2, :],
                                in0=dmat_sb[:],
                                scalar=amat_sb[:, ds(col, 1)],
                                in1=et[:],
                                op0=is_le,
                                op1=mult,
                            )
                            ptv = cur_pt2[:, j % 2, :]
                        if not island and j % 2 == 1:
                            pt2s.append(cur_pt2)
                        if j == 0:
                            nc.vector.tensor_copy(acc[:], ptv)
                        else:
                            nc.vector.tensor_add(acc[:], acc[:], ptv)
                    # den -> reciprocal row -> broadcast across partitions
                    dps = d_ps_pool.tile([1, CH], f32, name="dps")
                    nc.tensor.matmul(
                        dps[:], lhsT=ones_k_sb[:], rhs=acc[:], start=True,
                        stop=True,
                    )
                    # 1/den = exp(-ln(den)): ACT-only, avoids slow DVE recip
                    lden = f_pool.tile([1, CH], f32, tag="lden", name="lden")
                    nc.scalar.activation(lden[:], dps[:], Ln)
                    den_sb = f_pool.tile([1, CH], f16, tag="den", name="den")
                    nc.scalar.activation(den_sb[:], lden[:], Exp, scale=-1.0)
                    bps = b_ps_pool.tile([P, CH], f32, name="bps")
                    nc.tensor.matmul(
                        bps[:], lhsT=ones_r_sb[:], rhs=den_sb[:], start=True,
                        stop=True,
                    )
                    rec = f_pool.tile([P, CH], f16, tag="rec", name="rec")
                    nc.scalar.copy(rec[:], bps[:])
                    # prefetch next slot's V tiles while this slot's ctx runs
                    nvt = prefetch_v(slot + 1) if slot + 1 < NSLOT else None
                    n_mm = len(pt16s) + len(pt2s)
                    for o in range(8):
                        cps = c_ps_pool.tile([P, CH], f32, name="cps")
                        idx = 0
                        for vt, pt in zip(vtiles["i"], pt16s):
                            nc.tensor.matmul(
                                cps[:],
                                lhsT=vt[:, ts(o, P)],
                                rhs=pt[:],
                                start=(idx == 0),
                                stop=(idx == n_mm - 1),
                            )
                            idx += 1
                        for vt2, pt2 in zip(vtiles["p"], pt2s):
                            nc.tensor.matmul(
                                cps[:],
                                lhsT=vt2[:, :, ts(o, P)],
                                rhs=pt2[:],
                                start=(idx == 0),
                                stop=(idx == n_mm - 1),
                                perf_mode=DR,
                            )
                            idx += 1
                        ft = f_pool.tile([P, CH], f16, tag="ft", name="ft")
                        nc.vector.tensor_mul(ft[:], cps[:], rec[:])
                        eng = nc.sync if o % 2 == 0 else nc.scalar
                        eng.dma_start(
                            out=outT[ds(o * P, P), ts(slot, CH)], in_=ft[:]
                        )
                    vtiles = nvt

    nc.compile()
    return nc


def _get_program():
    global _PROGRAM
    if _PROGRAM is None:
        _PROGRAM = _build_program()
    return _PROGRAM


def _make_in_maps(x, W_query, W_key, W_value):
    import ml_dtypes

    f8 = ml_dtypes.float8_e4m3

    xT = np.ascontiguousarray(
        np.asarray(x, dtype=np.float32).transpose(0, 2, 1).astype(np.float16)
    )
    xT8 = xT.astype(f8)

    def tile_w(w, dt):
        # [d, o] -> [p, d_slab, o]
        wt = np.asarray(w, dtype=np.float32).T.astype(np.float16).astype(dt)
        return np.ascontiguousarray(wt.reshape(8, P, D).transpose(1, 0, 2))

    def tile_x(xt, nch):
        # [d, s] -> [chunk, p, d_slab, s_off]
        return np.ascontiguousarray(
            xt.reshape(8, P, nch, CH).transpose(2, 1, 0, 3)
        )

    w8 = {k: tile_w(w, f8) for k, w in
          (("wq8", W_query), ("wk8", W_key), ("wv8", W_value))}
    w16 = {k: tile_w(w, np.float16) for k, w in
           (("wq16", W_query), ("wk16", W_key), ("wv16", W_value))}
    dmat = (
        np.arange(P, dtype=np.float32)[:, None]
        - np.arange(CH, dtype=np.float32)[None, :]
    )
    dmat = np.ascontiguousarray(dmat.astype(np.float16))
    amat_h = []
    for h in range(2):
        a = np.zeros((P, 80), np.float16)
        for slot in range(NSLOT):
            cid = CHUNKS_H[h][slot]
            for j in range(NK[slot]):
                a[:, SLOTBASE[slot] + j] = CH * cid - P * j
        amat_h.append(a)
    ones_k = np.ones((P, 1), np.float16)
    ones_r = np.ones((1, P), np.float16)

    in_maps = []
    for core in range(8):
        b, h = core // 2, core % 2
        chunks = CHUNKS_H[h]
        xq8 = np.stack(
            [tile_x(xT8[b][:, c * CH : (c + 1) * CH], 1)[0] for c in chunks[1:]]
        )
        xq16 = tile_x(
            xT[b][:, chunks[0] * CH : (chunks[0] + 1) * CH], 1
        )[0]
        in_maps.append(
            {
                "xT8": tile_x(xT8[b], 8),
                "xc16": tile_x(xT[b][:, :CH], 1)[0],
                "xq8": xq8,
                "xq16": xq16,
                **w8,
                **w16,
                "amat": amat_h[h],
                "dmat": dmat,
                "ones_k": ones_k,
                "ones_r": ones_r,
            }
        )
    return in_maps


def _assemble(results):
    out = np.empty((B, S, D), np.float32)
    for core in range(8):
        b, h = core // 2, core % 2
        oT = np.asarray(results[core]["outT"]).astype(np.float32)  # [D, NQ]
        for slot, c in enumerate(CHUNKS_H[h]):
            out[b, c * CH : (c + 1) * CH, :] = oT[:, slot * CH : (slot + 1) * CH].T
    return out


def run(inputs, trace=False, trace_cores=None):
    """Run the kernel; returns (output, BassKernelResults)."""
    from concourse.bass_utils import run_bass_kernel_spmd

    nc = _get_program()
    in_maps = _make_in_maps(
        inputs["x"], inputs["W_query"], inputs["W_key"], inputs["W_value"]
    )
    kw = {}
    if trace:
        kw = dict(trace=True, trace_cores=trace_cores, stitch_traces=False)
    res = run_bass_kernel_spmd(nc, in_maps, list(range(8)), **kw)
    return _assemble(res.results), res


def kernel(x, W_query, W_key, W_value):
    out, _ = run({"x": x, "W_query": W_query, "W_key": W_key, "W_value": W_value})
    return out


# revision 9
# speedup vs baseline: 1.8527x; 1.0396x over previous
"""Causal single-head attention (B=4, S=4096, D=1024) on 8 TRN2 NeuronCores.

Sharding: core = (batch b, half h).  Each core computes attention output for
2048 queries of one batch: query chunks {0,3,4,7} (h=0) or {1,2,5,6} (h=1) of
8x512, which balances causal work.  Each core projects K^T/V for its full
batch (Q projections zippered in between the chunks); K^T and V both live
entirely in SBUF as fp8 (V is 4MB = 32KB/partition), so phase 2 needs no DMA
except the output.

All heavy matmuls run fp8e4m3 with perf_mode=DoubleRow (2 contraction slabs
per pass), except a small fp16 "island" covering keys 0..511 for slot-0
queries (chunks 0/1): early causal queries have peaked softmax, so fp8
quantization of scores/V would land directly on the output there.  The
island chunk (c=0) is projected LAST so its fp16 weights never gate startup.

  K^T/Q^T/V projections:  psum = sum_d2 WT[d2,:,:128].T @ x^T[d2,:,:]  (fp8 DR)
  scores^T[k,q]        :  psum = sum_o2 KT[o2,:,k128].T @ QT[o2,:,q512] (fp8 DR)
  P = exp(s*scale) * causal_mask   (mask = (iota_k - iota_q) <= a[slot,j]);
      causally-full tiles skip the mask: ACT writes exp straight to fp8
  den[1,q]             :  DVE-accumulate P tiles, then ones[k,1].T @ acc;
                          1/den = exp(-ln(den)) on ACT
  ctx^T[o,q]           :  one psum group per (slot,o) over all k-pairs
                          (fp8 DR); ft = psum * (1/den) -> fp16 out
"""

import sys

for _p in ("/opt/trn_rl_repo",):
    if _p not in sys.path:
        sys.path.insert(0, _p)

import numpy as np

B, S, D = 4, 4096, 1024
P = 128
CH = 512                       # query chunk
NSLOT = 4                      # chunks per core
NQ = NSLOT * CH                # queries per core
NK = [8, 16, 24, 32]           # k-tiles per slot (uniform across cores)
SLOTBASE = [0, 8, 24, 48]      # amat column base per slot
CHUNKS_H = [[0, 3, 4, 7], [1, 2, 5, 6]]
SCALE = 1.0 / 32.0             # 1/sqrt(D)

_PROGRAM = None


def _build_program():
    import concourse.bass as bass
    import concourse.tile as tile
    import concourse.mybir as mybir
    from concourse import bacc
    from concourse.bass import ds, ts

    f32 = mybir.dt.float32
    f16 = mybir.dt.float16
    f8 = mybir.dt.float8e4
    DR = mybir.MatmulPerfMode.DoubleRow

    nc = bacc.Bacc(trn_type="TRN2", target_bir_lowering=False, debug=False,
                   num_devices=8)

    xT8 = nc.declare_dram_parameter("xT8", [8, P, 8, CH], f8, isOutput=False)
    xc16d = nc.declare_dram_parameter("xc16", [P, 8, CH], f16, isOutput=False)
    xq8d = nc.declare_dram_parameter("xq8", [3, P, 8, CH], f8, isOutput=False)
    xq16d = nc.declare_dram_parameter("xq16", [P, 8, CH], f16, isOutput=False)
    wq8d = nc.declare_dram_parameter("wq8", [P, 8, D], f8, isOutput=False)
    wk8d = nc.declare_dram_parameter("wk8", [P, 8, D], f8, isOutput=False)
    wv8d = nc.declare_dram_parameter("wv8", [P, 8, D], f8, isOutput=False)
    wq16d = nc.declare_dram_parameter("wq16", [P, 8, D], f16, isOutput=False)
    wk16d = nc.declare_dram_parameter("wk16", [P, 8, D], f16, isOutput=False)
    wv16d = nc.declare_dram_parameter("wv16", [P, 8, D], f16, isOutput=False)
    amat = nc.declare_dram_parameter("amat", [P, 80], f16, isOutput=False)
    dmat = nc.declare_dram_parameter("dmat", [P, CH], f16, isOutput=False)
    ones_k = nc.declare_dram_parameter("ones_k", [P, 1], f16, isOutput=False)
    ones_r = nc.declare_dram_parameter("ones_r", [1, P], f16, isOutput=False)
    outT = nc.declare_dram_parameter("outT", [D, NQ], f16, isOutput=True)

    H = S // 4  # 1024: columns per resident K^T piece

    Exp = mybir.ActivationFunctionType.Exp
    Ln = mybir.ActivationFunctionType.Ln
    is_le = mybir.AluOpType.is_le
    mult = mybir.AluOpType.mult

    with tile.TileContext(nc, pool_alloc_mode="queue") as tc:
        with (
            tc.tile_pool(name="kt", bufs=1) as kt_pool,
            tc.tile_pool(name="qt", bufs=1) as qt_pool,
            tc.tile_pool(name="vs", bufs=1) as vs_pool,
            tc.tile_pool(name="const", bufs=1) as const_pool,
        ):
            KTp = [
                kt_pool.tile([P, 8, H], f8, tag=f"kt{i}", name=f"KTp{i}")
                for i in range(4)
            ]
            KT16 = kt_pool.tile([P, 8, CH], f16, tag="kt16", name="KT16")
            QTs = [
                qt_pool.tile([P, 8, CH], f8, tag=f"qt{i}", name=f"QTs{i}")
                for i in range(NSLOT)
            ]
            QT16 = qt_pool.tile([P, 8, CH], f16, tag="qt16", name="QT16")
            # V resident in SBUF: fp8 k-pair layout + fp16 island (keys 0..511)
            vsb = vs_pool.tile([P, 16, 2, D], f8, tag="vsb", name="vsb")
            v16 = vs_pool.tile([P, 4, D], f16, tag="v16", name="v16")
            dmat_sb = const_pool.tile([P, CH], f16, tag="dmat")
            amat_sb = const_pool.tile([P, 80], f16, tag="amat")
            ones_k_sb = const_pool.tile([P, 1], f16, tag="onesk")
            ones_r_sb = const_pool.tile([1, P], f16, tag="onesr")
            nc.gpsimd.dma_start(out=dmat_sb[:], in_=dmat[:])
            nc.gpsimd.dma_start(out=amat_sb[:], in_=amat[:])
            nc.gpsimd.dma_start(out=ones_k_sb[:], in_=ones_k[:])
            nc.gpsimd.dma_start(out=ones_r_sb[:], in_=ones_r[:])

            # ---------- Phase 0+1: local projections (K, V, Q zippered) ----
            with (
                tc.tile_pool(name="w0", bufs=1) as w_pool,
                tc.tile_pool(name="xc", bufs=2) as x_pool,
                tc.tile_pool(name="xq", bufs=2) as xq_pool,
                tc.tile_pool(name="ps0", bufs=4, space="PSUM") as ps_pool,
            ):
                wk8 = w_pool.tile([P, 8, D], f8, tag="wk8")
                wv8 = w_pool.tile([P, 8, D], f8, tag="wv8")
                wq8 = w_pool.tile([P, 8, D], f8, tag="wq8")
                # wa16 carries wq16 (Q island) then is reloaded with wk16;
                # x16 carries xq16 then xc16.  wv16 has its own tile.
                wa16 = w_pool.tile([P, 8, D], f16, tag="wa16")
                wv16 = w_pool.tile([P, 8, D], f16, tag="wv16")
                x16 = w_pool.tile([P, 8, CH], f16, tag="x16")
                # striped initial loads for the first fp8 chunk
                for d2 in range(4):
                    eng = nc.sync if d2 < 2 else nc.scalar
                    eng.dma_start(
                        out=wk8[:, ds(2 * d2, 2), :],
                        in_=wk8d[:, ds(2 * d2, 2), :],
                    )

                def load_xq(s):
                    xq = xq_pool.tile([P, 8, CH], f8, tag="xq", name=f"xq{s}")
                    nc.scalar.dma_start(out=xq[:], in_=xq8d[s - 1])
                    return xq

                xq_pending = {}

                def proj_q8(s):
                    xq = xq_pending[s]
                    for o in range(8):
                        ps = ps_pool.tile([P, CH], f32, tag="ps", name="psq")
                        for d2 in range(4):
                            nc.tensor.matmul(
                                ps[:],
                                lhsT=wq8[:, ds(2 * d2, 2), ts(o, P)],
                                rhs=xq[:, ds(2 * d2, 2), :],
                                start=(d2 == 0),
                                stop=(d2 == 3),
                                perf_mode=DR,
                            )
                        nc.vector.tensor_copy(QTs[s][:, o, :], ps[:])

                def proj_q16():
                    # slot-0 Q in fp16 (wa16 = wq16, x16 = xq16), dual-cast
                    for o in range(8):
                        ps = ps_pool.tile([P, CH], f32, tag="ps", name="psq6")
                        for d in range(8):
                            nc.tensor.matmul(
                                ps[:],
                                lhsT=wa16[:, d, ts(o, P)],
                                rhs=x16[:, d, :],
                                start=(d == 0),
                                stop=(d == 7),
                            )
                        nc.vector.tensor_copy(QT16[:, o, :], ps[:])
                        nc.scalar.copy(QTs[0][:, o, :], ps[:])

                def proj_kv16():
                    # chunk 0 in fp16 (wa16 = wk16, x16 = xc16), dual-cast
                    for o in range(8):
                        ps = ps_pool.tile([P, CH], f32, tag="ps", name="psk6")
                        for d in range(8):
                            nc.tensor.matmul(
                                ps[:],
                                lhsT=wa16[:, d, ts(o, P)],
                                rhs=x16[:, d, :],
                                start=(d == 0),
                                stop=(d == 7),
                            )
                        nc.vector.tensor_copy(KT16[:, o, :], ps[:])
                        nc.scalar.copy(KTp[0][:, o, ds(0, CH)], ps[:])
                    for kt_i in range(4):
                        for oh in range(2):
                            ps = ps_pool.tile([P, CH], f32, tag="ps", name="psv6")
                            for d in range(8):
                                nc.tensor.matmul(
                                    ps[:],
                                    lhsT=x16[:, d, ts(kt_i, P)],
                                    rhs=wv16[:, d, ts(oh, CH)],
                                    start=(d == 0),
                                    stop=(d == 7),
                                )
                            nc.scalar.copy(v16[:, kt_i, ts(oh, CH)], ps[:])
                            nc.vector.tensor_copy(
                                vsb[:, kt_i // 2, kt_i % 2, ts(oh, CH)], ps[:]
                            )

                # chunk 0 (fp16 island) LAST: its weights stream in while the
                # fp8 chunks compute.  Q slots zippered after iters 1..4.
                c_order = [1, 2, 3, 4, 5, 6, 7, 0]
                q_sched = {1: 1, 2: 2, 3: 3, 4: 0}
                for it, c in enumerate(c_order):
                    if c == 0:
                        proj_kv16()
                    else:
                        xc = x_pool.tile([P, 8, CH], f8, tag="xc", name=f"xc{c}")
                        if it == 0:
                            for sp in range(4):
                                eng = nc.gpsimd if sp % 2 == 0 else nc.sync
                                eng.dma_start(
                                    out=xc[:, ds(sp * 2, 2), :],
                                    in_=xT8[c][:, ds(sp * 2, 2), :],
                                )
                        else:
                            nc.sync.dma_start(out=xc[:], in_=xT8[c])
                        for o in range(8):
                            ps = ps_pool.tile([P, CH], f32, tag="ps", name="psk")
                            for d2 in range(4):
                                nc.tensor.matmul(
                                    ps[:],
                                    lhsT=wk8[:, ds(2 * d2, 2), ts(o, P)],
                                    rhs=xc[:, ds(2 * d2, 2), :],
                                    start=(d2 == 0),
                                    stop=(d2 == 3),
                                    perf_mode=DR,
                                )
                            if o % 2 == 0:
                                nc.vector.tensor_copy(
                                    KTp[c // 2][:, o, ds((c % 2) * CH, CH)],
                                    ps[:],
                                )
                            else:
                                nc.scalar.copy(
                                    KTp[c // 2][:, o, ds((c % 2) * CH, CH)],
                                    ps[:],
                                )
                        if it == 0:
                            # deferred loads, enqueued between chunk-1's K and
                            # V work (wv8 must precede the V copies in the
                            # ACT queue to avoid a trigger deadlock)
                            for d2 in range(4):
                                nc.scalar.dma_start(
                                    out=wv8[:, ds(2 * d2, 2), :],
                                    in_=wv8d[:, ds(2 * d2, 2), :],
                                )
                            nc.scalar.dma_start(out=wq8[:], in_=wq8d[:])
                            xq_pending[1] = load_xq(1)
                            xq_pending[2] = load_xq(2)
                            nc.gpsimd.dma_start(out=wa16[:], in_=wq16d[:])
                            nc.gpsimd.dma_start(out=x16[:], in_=xq16d[:])
                        for kt_i in range(4):
                            j = c * 4 + kt_i
                            for oh in range(2):
                                ps = ps_pool.tile([P, CH], f32, tag="ps", name="psv")
                                for d2 in range(4):
                                    nc.tensor.matmul(
                                        ps[:],
                                        lhsT=xc[:, ds(2 * d2, 2), ts(kt_i, P)],
                                        rhs=wv8[:, ds(2 * d2, 2), ts(oh, CH)],
                                        start=(d2 == 0),
                                        stop=(d2 == 3),
                                        perf_mode=DR,
                                    )
                                nc.scalar.copy(
                                    vsb[:, j // 2, j % 2, ts(oh, CH)], ps[:]
                                )
                    sq = q_sched.get(it)
                    if sq is not None:
                        if sq == 0:
                            proj_q16()
                            # island loads: wa16 <- wk16, x16 <- xc16 (tile
                            # reuse; WAR deps handled), consumed at it==7
                            for sp in range(4):
                                nc.sync.dma_start(
                                    out=wa16[:, ds(2 * sp, 2), :],
                                    in_=wk16d[:, ds(2 * sp, 2), :],
                                )
                            nc.gpsimd.dma_start(out=x16[:], in_=xc16d[:])
                        else:
                            proj_q8(sq)
                            if sq == 2:
                                xq_pending[3] = load_xq(3)
                            if sq == 3:
                                for sp in range(4):
                                    nc.scalar.dma_start(
                                        out=wv16[:, ds(2 * sp, 2), :],
                                        in_=wv16d[:, ds(2 * sp, 2), :],
                                    )

            # ---------------- Phase 2: attention ---------------------------
            # Per slot: all score k-tiles (P tiles + den accumulate), then
            # ctx as ONE psum group per o; ft multiplies read psum directly.
            # Tiles causally full on BOTH halves skip the mask STT: ACT
            # writes exp() straight into the fp8 P pair tile.
            CLEAN_NK = [
                min(4 * CHUNKS_H[0][s], 4 * CHUNKS_H[1][s], NK[s])
                for s in range(NSLOT)
            ]
            with (
                tc.tile_pool(name="pt", bufs=20) as p_pool,
                tc.tile_pool(name="p6", bufs=4) as p16_pool,
                tc.tile_pool(name="et", bufs=3) as e_pool,
                tc.tile_pool(name="fo", bufs=6) as f_pool,
                tc.tile_pool(name="dsb", bufs=2) as den_pool,
                tc.tile_pool(name="pss", bufs=3, space="PSUM") as s_ps_pool,
                tc.tile_pool(name="psc", bufs=3, space="PSUM") as c_ps_pool,
                tc.tile_pool(name="psd", bufs=1, space="PSUM") as d_ps_pool,
                tc.tile_pool(name="psb", bufs=1, space="PSUM") as b_ps_pool,
            ):
                for slot in range(NSLOT):
                    nk = NK[slot]
                    acc = den_pool.tile([P, CH], f16, tag="acc", name="acc")
                    pt16s = []
                    pt2s = []
                    cur_pt2 = None
                    for j in range(nk):
                        island = slot == 0 and j < 4
                        clean = j < CLEAN_NK[slot]
                        sps = s_ps_pool.tile([P, CH], f32, name="sps")
                        if island:
                            for o in range(8):
                                nc.tensor.matmul(
                                    sps[:],
                                    lhsT=KT16[:, o, ds(j * P, P)],
                                    rhs=QT16[:, o, :],
                                    start=(o == 0),
                                    stop=(o == 7),
                                )
                        else:
                            for o2 in range(4):
                                nc.tensor.matmul(
                                    sps[:],
                                    lhsT=KTp[j // 8][
                                        :, ds(2 * o2, 2), ds((j % 8) * P, P)
                                    ],
                                    rhs=QTs[slot][:, ds(2 * o2, 2), :],
                                    start=(o2 == 0),
                                    stop=(o2 == 3),
                                    perf_mode=DR,
                                )
                        if not island and j % 2 == 0:
                            cur_pt2 = p_pool.tile([P, 2, CH], f8, tag="pt",
                                                  name="pt2")
                        col = SLOTBASE[slot] + j
                        if island:
                            et = e_pool.tile([P, CH], f16, tag="et", name="et")
                            nc.scalar.activation(et[:], sps[:], Exp, scale=SCALE)
                            pt = p16_pool.tile([P, CH], f16, tag="pt16",
                                               name="pt16")
                            nc.vector.scalar_tensor_tensor(
                                out=pt[:],
                                in0=dmat_sb[:],
                                scalar=amat_sb[:, ds(col, 1)],
                                in1=et[:],
                                op0=is_le,
                                op1=mult,
                            )
                            pt16s.append(pt)
                            ptv = pt[:]
                        elif clean:
                            nc.scalar.activation(
                                cur_pt2[:, j % 2, :], sps[:], Exp, scale=SCALE
                            )
                            ptv = cur_pt2[:, j % 2, :]
                        else:
                            et = e_pool.tile([P, CH], f16, tag="et", name="et")
                            nc.scalar.activation(et[:], sps[:], Exp, scale=SCALE)
                            nc.vector.scalar_tensor_tensor(
                                out=cur_pt2[:, j % 2, :],
                                in0=dmat_sb[:],
                                scalar=amat_sb[:, ds(col, 1)],
                                in1=et[:],
                                op0=is_le,
                                op1=mult,
                            )
                            ptv = cur_pt2[:, j % 2, :]
                        if not island and j % 2 == 1:
                            pt2s.append(cur_pt2)
                        if j == 0:
                            nc.vector.tensor_copy(acc[:], ptv)
                        else:
                            nc.vector.tensor_add(acc[:], acc[:], ptv)
                    # den -> 1/den = exp(-ln(den)) -> broadcast; the o=0 ctx
                    # group is emitted between den and bps so the PE never
                    # waits on the ACT chain.
                    dps = d_ps_pool.tile([1, CH], f32, name="dps")
                    nc.tensor.matmul(
                        dps[:], lhsT=ones_k_sb[:], rhs=acc[:], start=True,
                        stop=True,
                    )
                    lden = f_pool.tile([1, CH], f32, tag="lden", name="lden")
                    nc.scalar.activation(lden[:], dps[:], Ln)
                    den_sb = f_pool.tile([1, CH], f16, tag="den", name="den")
                    nc.scalar.activation(den_sb[:], lden[:], Exp, scale=-1.0)

                    n_mm = len(pt16s) + len(pt2s)

                    def ctx_group(o, slot=slot, n_mm=n_mm, pt16s=pt16s,
                                  pt2s=pt2s):
                        cps = c_ps_pool.tile([P, CH], f32, name="cps")
                        idx = 0
                        for jj, pt in enumerate(pt16s):
                            nc.tensor.matmul(
                                cps[:],
                                lhsT=v16[:, jj, ts(o, P)],
                                rhs=pt[:],
                                start=(idx == 0),
                                stop=(idx == n_mm - 1),
                            )
                            idx += 1
                        p0 = 2 if slot == 0 else 0
                        for pi, pt2 in enumerate(pt2s):
                            nc.tensor.matmul(
                                cps[:],
                                lhsT=vsb[:, p0 + pi, :, ts(o, P)],
                                rhs=pt2[:],
                                start=(idx == 0),
                                stop=(idx == n_mm - 1),
                                perf_mode=DR,
                            )
                            idx += 1
                        return cps

                    cps0 = ctx_group(0)
                    bps = b_ps_pool.tile([P, CH], f32, name="bps")
                    nc.tensor.matmul(
                        bps[:], lhsT=ones_r_sb[:], rhs=den_sb[:], start=True,
                        stop=True,
                    )
                    rec = f_pool.tile([P, CH], f16, tag="rec", name="rec")
                    nc.scalar.copy(rec[:], bps[:])
                    ft = f_pool.tile([P, CH], f16, tag="ft", name="ft")
                    nc.vector.tensor_mul(ft[:], cps0[:], rec[:])
                    nc.sync.dma_start(
                        out=outT[ds(0, P), ts(slot, CH)], in_=ft[:]
                    )
                    for o in range(1, 8):
                        cps = ctx_group(o)
                        ft = f_pool.tile([P, CH], f16, tag="ft", name="ft")
                        nc.vector.tensor_mul(ft[:], cps[:], rec[:])
                        eng = nc.sync if o % 2 == 0 else nc.scalar
                        eng.dma_start(
                            out=outT[ds(o * P, P), ts(slot, CH)], in_=ft[:]
                        )

    nc.compile()
    return nc


def _get_program():
    global _PROGRAM
    if _PROGRAM is None:
        _PROGRAM = _build_program()
    return _PROGRAM


def _make_in_maps(x, W_query, W_key, W_value):
    import ml_dtypes

    f8 = ml_dtypes.float8_e4m3

    xT = np.ascontiguousarray(
        np.asarray(x, dtype=np.float32).transpose(0, 2, 1).astype(np.float16)
    )
    xT8 = xT.astype(f8)

    def tile_w(w, dt):
        # [d, o] -> [p, d_slab, o]
        wt = np.asarray(w, dtype=np.float32).T.astype(np.float16).astype(dt)
        return np.ascontiguousarray(wt.reshape(8, P, D).transpose(1, 0, 2))

    def tile_x(xt, nch):
        # [d, s] -> [chunk, p, d_slab, s_off]
        return np.ascontiguousarray(
            xt.reshape(8, P, nch, CH).transpose(2, 1, 0, 3)
        )

    w8 = {k: tile_w(w, f8) for k, w in
          (("wq8", W_query), ("wk8", W_key), ("wv8", W_value))}
    w16 = {k: tile_w(w, np.float16) for k, w in
           (("wq16", W_query), ("wk16", W_key), ("wv16", W_value))}
    dmat = (
        np.arange(P, dtype=np.float32)[:, None]
        - np.arange(CH, dtype=np.float32)[None, :]
    )
    dmat = np.ascontiguousarray(dmat.astype(np.float16))
    amat_h = []
    for h in range(2):
        a = np.zeros((P, 80), np.float16)
        for slot in range(NSLOT):
            cid = CHUNKS_H[h][slot]
            for j in range(NK[slot]):
                a[:, SLOTBASE[slot] + j] = CH * cid - P * j
        amat_h.append(a)
    ones_k = np.ones((P, 1), np.float16)
    ones_r = np.ones((1, P), np.float16)

    in_maps = []
    for core in range(8):
        b, h = core // 2, core % 2
        chunks = CHUNKS_H[h]
        xq8 = np.stack(
            [tile_x(xT8[b][:, c * CH : (c + 1) * CH], 1)[0] for c in chunks[1:]]
        )
        xq16 = tile_x(
            xT[b][:, chunks[0] * CH : (chunks[0] + 1) * CH], 1
        )[0]
        in_maps.append(
            {
                "xT8": tile_x(xT8[b], 8),
                "xc16": tile_x(xT[b][:, :CH], 1)[0],
                "xq8": xq8,
                "xq16": xq16,
                **w8,
                **w16,
                "amat": amat_h[h],
                "dmat": dmat,
                "ones_k": ones_k,
                "ones_r": ones_r,
            }
        )
    return in_maps


def _assemble(results):
    out = np.empty((B, S, D), np.float32)
    for core in range(8):
        b, h = core // 2, core % 2
        oT = np.asarray(results[core]["outT"]).astype(np.float32)  # [D, NQ]
        for slot, c in enumerate(CHUNKS_H[h]):
            out[b, c * CH : (c + 1) * CH, :] = oT[:, slot * CH : (slot + 1) * CH].T
    return out


def run(inputs, trace=False, trace_cores=None):
    """Run the kernel; returns (output, BassKernelResults)."""
    from concourse.bass_utils import run_bass_kernel_spmd

    nc = _get_program()
    in_maps = _make_in_maps(
        inputs["x"], inputs["W_query"], inputs["W_key"], inputs["W_value"]
    )
    kw = {}
    if trace:
        kw = dict(trace=True, trace_cores=trace_cores, stitch_traces=False)
    res = run_bass_kernel_spmd(nc, in_maps, list(range(8)), **kw)
    return _assemble(res.results), res


def kernel(x, W_query, W_key, W_value):
    out, _ = run({"x": x, "W_query": W_query, "W_key": W_key, "W_value": W_value})
    return out
